# revision 8
# baseline (speedup 1.0000x reference)
"""Trainium2 Bass kernel for nn_ConvMultiHeadAttn.

Reference computation (per batch b):
  qkv = x @ Wqkv ; q,k1,k2,v = split(qkv)            [L, 4D]
  s1 = q @ k1^T ; s2 = q @ k2^T   (per head)         [H, L, L]
  attn = where(qmask_q == qmask_k, s1, s2)
  attn = where(mask_k, attn, -1e9) + dis             dis = -(shift*(tq-tk)^2 + bias_p)
  out = softmax(attn) @ v ; out = out @ Wfc + bfc

Strategy: data-parallel over batch (2 batches per NeuronCore, 8 cores, no
collectives). Scores are computed k-major (scores^T[k, q]) so the softmax
denominator and the attn@v contraction are matmuls.  The qmask select is a
single predicated copy on DVE; the mask bias rides the exp() per-partition
bias; the Gaussian bias is folded in as exp(dis) (host precomputed) via one
elementwise multiply on GpSimd.  Softmax needs no max pass (logits bounded,
exp stored in fp32->bf16) and the denominator comes free as a ones column in
the attn@v matmul.  fp16 operands on the q/k side (score accuracy), bf16 on
the v/fc side.
"""

import numpy as np
import ml_dtypes

import concourse.bass as bass
import concourse.bacc as bacc
import concourse.mybir as mybir
import concourse.tile as tile
from concourse.bass_utils import run_bass_kernel_spmd
from concourse.masks import make_identity

B, L, D, H = 16, 512, 1024, 16
DH = D // H            # 64
NCORES = 8
BPC = B // NCORES      # batches per core
KC = L // 128          # 4 token chunks
DCH = D // 128         # 8 d-model chunks
NEG = -1e9

F16 = mybir.dt.float16
BF16 = mybir.dt.bfloat16
F32 = mybir.dt.float32
EXP = mybir.ActivationFunctionType.Exp


def _build_bass():
    nc = bacc.Bacc(trn_type="TRN2")
    xb = nc.dram_tensor("xb", [BPC, L, D], F16, kind="ExternalInput")
    wqkv = nc.dram_tensor("wqkv", [D, 4 * D], F16, kind="ExternalInput")
    wfc = nc.dram_tensor("wfc", [D, D], BF16, kind="ExternalInput")
    bfc = nc.dram_tensor("bfc", [D], F32, kind="ExternalInput")
    same = nc.dram_tensor("same", [BPC, L, L], mybir.dt.uint8, kind="ExternalInput")
    gdis = nc.dram_tensor("gdis", [L, L], BF16, kind="ExternalInput")
    kbias = nc.dram_tensor("kbias", [128, BPC * KC], F32, kind="ExternalInput")
    outp = nc.dram_tensor("outp", [BPC, L, D], F32, kind="ExternalOutput")

    with tile.TileContext(nc) as tc:
        with (
            tc.tile_pool(name="w", bufs=1) as wpool,
            tc.tile_pool(name="a", bufs=1) as apool,
            tc.tile_pool(name="x", bufs=2) as xpool,
            tc.tile_pool(name="e", bufs=4) as epool,
            tc.tile_pool(name="d", bufs=2) as dpool,
            tc.tile_pool(name="o", bufs=4) as opool,
            tc.tile_pool(name="mm", bufs=2, space="PSUM") as mmps,
            tc.tile_pool(name="sc", bufs=4, space="PSUM") as scps,
            tc.tile_pool(name="av", bufs=2, space="PSUM") as avps,
        ):
            # resident weights / constants
            wq_sb = wpool.tile([128, DCH, 4 * D], F16)
            nc.sync.dma_start(
                out=wq_sb, in_=wqkv[:, :].rearrange("(c p) f -> p c f", p=128)
            )
            wf_sb = wpool.tile([128, DCH, D], BF16)
            nc.sync.dma_start(
                out=wf_sb, in_=wfc[:, :].rearrange("(c p) f -> p c f", p=128)
            )
            bf_sb = wpool.tile([128, D], F32)
            nc.sync.dma_start(out=bf_sb, in_=bfc[:].unsqueeze(0).broadcast_to([128, D]))
            kb_sb = wpool.tile([128, BPC * KC], F32)
            nc.sync.dma_start(out=kb_sb, in_=kbias[:, :])
            gd_sb = wpool.tile([128, KC, L], BF16)
            nc.sync.dma_start(
                out=gd_sb, in_=gdis[:, :].rearrange("(c p) q -> p c q", p=128)
            )
            ident = wpool.tile([128, 128], F16)
            make_identity(nc, ident)

            # round-robin PSUM->SBUF evacuation between DVE and ACT
            rr = [0]

            def evac(dst, src):
                if rr[0] % 2 == 0:
                    nc.vector.tensor_copy(out=dst, in_=src)
                else:
                    nc.scalar.copy(out=dst, in_=src)
                rr[0] += 1

            for b in range(BPC):
                sm = xpool.tile([128, KC, L], mybir.dt.uint8, tag="sm")
                nc.sync.dma_start(
                    out=sm, in_=same[b, :, :].rearrange("(c p) q -> p c q", p=128)
                )
                # x tok-major load, then x^T (d on partitions) via PE transpose
                xin = xpool.tile([128, KC, D], F16, tag="xin")
                nc.sync.dma_start(
                    out=xin, in_=xb[b, :, :].rearrange("(t p) d -> p t d", p=128)
                )
                xT = xpool.tile([128, DCH, L], F16, tag="xT", bufs=1)
                for c in range(DCH):
                    for t in range(KC):
                        tp = mmps.tile([128, 128], F16, tag="mm")
                        nc.tensor.transpose(
                            tp, xin[:, t, c * 128 : (c + 1) * 128], ident
                        )
                        evac(xT[:, c, t * 128 : (t + 1) * 128], tp)

                qt = apool.tile([128, DCH, L], F16, tag="qt")
                kt1 = apool.tile([128, DCH, L], F16, tag="kt1")
                kt2 = apool.tile([128, DCH, L], F16, tag="kt2")
                vsb = apool.tile([128, KC, H, DH + 2], BF16, tag="vsb")
                ctxT = apool.tile([128, DCH, L], BF16, tag="ctxT")
                nc.vector.memset(vsb[:, :, :, DH : DH + 1], 1.0)

                # qkv projections.  Q/K1/K2 feature-major (k-major for scores),
                # V token-major (natural) for the attn@v stationary operand.
                for which, foff in ((qt, 0), (kt1, D), (kt2, 2 * D)):
                    for j in range(8):
                        ps = mmps.tile([128, L], F32, tag="mm")
                        for c in range(DCH):
                            nc.tensor.matmul(
                                ps,
                                lhsT=wq_sb[:, c, foff + j * 128 : foff + (j + 1) * 128],
                                rhs=xT[:, c, :],
                                start=(c == 0),
                                stop=(c == DCH - 1),
                            )
                        evac(which[:, j, :], ps)
                for t in range(KC):
                    for vc in range(2):
                        ps = mmps.tile([128, L], F32, tag="mm")
                        for c in range(DCH):
                            nc.tensor.matmul(
                                ps,
                                lhsT=xT[:, c, t * 128 : (t + 1) * 128],
                                rhs=wq_sb[:, c, 3 * D + vc * 512 : 3 * D + (vc + 1) * 512],
                                start=(c == 0),
                                stop=(c == DCH - 1),
                            )
                        evac(
                            vsb[:, t, vc * 8 : (vc + 1) * 8, 0:DH],
                            ps[:, :].rearrange("p (h e) -> p h e", h=8),
                        )

                # attention, two heads (one feature chunk) at a time
                Dall = dpool.tile([16, L], F32, tag="Dall")
                for hp in range(8):
                    h0, h1 = 2 * hp, 2 * hp + 1
                    av_a = avps.tile([DH + 1, L], F32, tag="av")
                    av_b = avps.tile([DH + 1, L], F32, tag="av")
                    for kc in range(KC):
                        ks = slice(kc * 128, (kc + 1) * 128)
                        s1a = scps.tile([128, L], F32, tag="sc")
                        s1b = scps.tile([128, L], F32, tag="sc")
                        s2a = scps.tile([128, L], F32, tag="sc")
                        s2b = scps.tile([128, L], F32, tag="sc")
                        # scores^T[k, q]; even head on PE rows 0-63, odd on 64-127
                        nc.tensor.matmul(s1a, lhsT=kt1[0:64, hp, ks], rhs=qt[0:64, hp, :])
                        nc.tensor.matmul(s1b, lhsT=kt1[64:128, hp, ks], rhs=qt[64:128, hp, :])
                        nc.tensor.matmul(s2a, lhsT=kt2[0:64, hp, ks], rhs=qt[0:64, hp, :])
                        nc.tensor.matmul(s2b, lhsT=kt2[64:128, hp, ks], rhs=qt[64:128, hp, :])
                        # attn = where(same, s1, s2), in place in s2
                        nc.vector.copy_predicated(out=s2a, mask=sm[:, kc, :], data=s1a)
                        nc.vector.copy_predicated(out=s2b, mask=sm[:, kc, :], data=s1b)
                        # exp(attn + keymask bias); then * exp(dis) on gpsimd
                        kb = kb_sb[:, b * KC + kc : b * KC + kc + 1]
                        ea = epool.tile([128, L], BF16, tag="ea", bufs=3)
                        eb = epool.tile([128, L], BF16, tag="eb", bufs=3)
                        nc.scalar.activation(out=ea, in_=s2a, func=EXP, bias=kb, scale=1.0)
                        nc.scalar.activation(out=eb, in_=s2b, func=EXP, bias=kb, scale=1.0)
                        pa = epool.tile([128, L], BF16, tag="pa", bufs=3)
                        pb = epool.tile([128, L], BF16, tag="pb", bufs=3)
                        nc.gpsimd.tensor_mul(pa, ea, gd_sb[:, kc, :])
                        nc.gpsimd.tensor_mul(pb, eb, gd_sb[:, kc, :])
                        # attn @ v with a ones column -> softmax denominator row
                        nc.tensor.matmul(
                            av_a, lhsT=vsb[:, kc, h0, 0 : DH + 1], rhs=pa,
                            start=(kc == 0), stop=(kc == KC - 1),
                        )
                        nc.tensor.matmul(
                            av_b, lhsT=vsb[:, kc, h1, 0 : DH + 1], rhs=pb,
                            start=(kc == 0), stop=(kc == KC - 1),
                        )
                    # evacuate ctx^T (unnormalized); odd head shifts to rows 64-127
                    nc.scalar.copy(out=ctxT[0:64, hp, :], in_=av_a[0:DH, :])
                    cu = epool.tile([64, L], BF16, tag="cu")
                    nc.scalar.copy(out=cu, in_=av_b[0:DH, :])
                    nc.sync.dma_start(out=ctxT[64:128, hp, :], in_=cu)
                    # denominator rows -> Dall
                    da = dpool.tile([65, L], F32, tag="da")
                    db = dpool.tile([65, L], F32, tag="db")
                    nc.vector.tensor_copy(out=da[64:65, :], in_=av_a[DH : DH + 1, :])
                    nc.vector.tensor_copy(out=db[64:65, :], in_=av_b[DH : DH + 1, :])
                    nc.sync.dma_start(out=Dall[h0 : h0 + 1, :], in_=da[64:65, :])
                    nc.sync.dma_start(out=Dall[h1 : h1 + 1, :], in_=db[64:65, :])

                # softmax normalization: ctxT *= broadcast(1/denom)
                Rf = dpool.tile([16, L], F32, tag="Rf")
                nc.vector.reciprocal_approx_fast(out=Rf, in_=Dall)
                Rb16 = dpool.tile([16, L], BF16, tag="Rb16")
                nc.vector.tensor_copy(out=Rb16, in_=Rf)
                for hp in range(8):
                    rb = opool.tile([128, L], BF16, tag="rb")
                    nc.sync.dma_start(
                        out=rb,
                        in_=Rb16[2 * hp : 2 * hp + 2, :].unsqueeze(1).broadcast_to([2, 64, L]),
                    )
                    nc.vector.tensor_mul(ctxT[:, hp, :], ctxT[:, hp, :], rb)

                # fc + bias
                for t in range(KC):
                    for oc in range(2):
                        ps = mmps.tile([128, 512], F32, tag="mm")
                        for c in range(DCH):
                            nc.tensor.matmul(
                                ps,
                                lhsT=ctxT[:, c, t * 128 : (t + 1) * 128],
                                rhs=wf_sb[:, c, oc * 512 : (oc + 1) * 512],
                                start=(c == 0),
                                stop=(c == DCH - 1),
                            )
                        ob = opool.tile([128, 512], F32, tag="ob")
                        nc.vector.tensor_add(ob, ps, bf_sb[:, oc * 512 : (oc + 1) * 512])
                        nc.sync.dma_start(
                            out=outp[b, t * 128 : (t + 1) * 128, oc * 512 : (oc + 1) * 512],
                            in_=ob,
                        )
    return nc


_NC_CACHE = None


def _get_nc():
    global _NC_CACHE
    if _NC_CACHE is None:
        nc = _build_bass()
        nc.finalize()
        _NC_CACHE = nc
    return _NC_CACHE


def kernel(x, mask, qmask, Wqkv, Wfc, bfc, shift, bias_p, use_Gaussian):
    bf16 = ml_dtypes.bfloat16
    x = np.asarray(x, dtype=np.float32)
    mask = np.asarray(mask)
    qmask = np.asarray(qmask)
    wq16 = np.ascontiguousarray(np.asarray(Wqkv, dtype=np.float32).astype(np.float16))
    wf16 = np.ascontiguousarray(np.asarray(Wfc, dtype=np.float32).astype(bf16))
    bfc32 = np.ascontiguousarray(np.asarray(bfc, dtype=np.float32))
    shift_v = float(np.asarray(shift, dtype=np.float64).reshape(-1)[0])
    bias_v = float(np.asarray(bias_p, dtype=np.float64).reshape(-1)[0])
    ug = bool(np.asarray(use_Gaussian).reshape(-1)[0])

    x16 = x.astype(np.float16)
    same_b = (qmask[:, :, None] == qmask[:, None, :]).astype(np.uint8)  # [B, L, L]
    idx = np.arange(L, dtype=np.float64)
    if ug:
        dis = -(shift_v * (idx[:, None] - idx[None, :]) ** 2 + bias_v)
    else:
        dis = np.zeros((L, L), dtype=np.float64)
    gdis_b = np.exp(dis).astype(bf16)
    keyadd = np.where(mask != 0, 0.0, NEG).astype(np.float32)  # [B, L]
    kb_all = keyadd.reshape(B, KC, 128).transpose(2, 0, 1)  # [128, B, KC]

    nc = _get_nc()
    in_maps = []
    for core in range(NCORES):
        bs = slice(core * BPC, (core + 1) * BPC)
        in_maps.append(
            {
                "xb": np.ascontiguousarray(x16[bs]),
                "wqkv": wq16,
                "wfc": wf16,
                "bfc": bfc32,
                "same": np.ascontiguousarray(same_b[bs]),
                "gdis": gdis_b,
                "kbias": np.ascontiguousarray(
                    kb_all[:, bs, :].reshape(128, BPC * KC)
                ),
            }
        )
    res = run_bass_kernel_spmd(nc, in_maps, core_ids=list(range(NCORES)))
    kernel.last_perf = res
    out = np.concatenate([r["outp"] for r in res.results], axis=0)
    return np.ascontiguousarray(out.astype(np.float32))
